# revision 10
# baseline (speedup 1.0000x reference)
"""DeltaSynapse (gnn_message_passing) Trainium2 Bass kernel.

Computes I[b,o] = sum_e signs[e,o]*(W[e,o]*(1-f[e,o]) + Wlong[b,e,o]*f[e,o])
                  * Xpre[b,e,o],
with Xpre[b,e,o] = sum_d delaymap[d,e,o]*Xd[d,b,e]  (one-hot delay gather).

Strategy (8 NeuronCores): shard the postsynaptic axis o into 4 quarters of
512 and the presynaptic axis e into 2 halves of 1024; core (h,q) computes
the partial sum over its e-half for its o-quarter. The two e-half partials
are summed on the host (64KB) and the o-quarters concatenated.

The kernel is HBM-bound (38.25 MiB of f32 reads per core at ~380 GB/s
~= 105 us), so the schedule keeps the DMA stream saturated end to end:

  - Host hands each core pre-permuted shards so every load is an
    identity-mapped DMA with 8-16 KB contiguous runs per partition.
  - Aux tensors (Xd, W, STDP, signs) are issued FIRST; engines' static
    program order then computes packed spikes and A = sgn*W*(1-f),
    C = sgn*f before the first stream tile's elementwise work needs
    them, and the DMA queue is never idle behind compute.
  - Steady state streams delaymap (d-split halves) + Wlong per 128-e
    tile, cast f32->f16 in the DMA engines (gpsimd-issued SWDGE).
  - Xd is bit-packed once (packed[e,d] = sum_b 2^b*Xd[d,b,e]); per tile
    the one-hot delay select runs on the PE as sum_d diag(packed[:,d])
    @ dmap[d], landing all 8 per-batch spike bits as an exact integer
    Pi in PSUM. Masks m[b] = (Pi>>b)&1 extract on DVE in b-pairs, cast
    i16->f16 on Scalar, and t[b] = (A + C*Wlong[b])*m[b] applies on DVE
    with out-of-place fp16 ops (2x perf mode); one-hot-column matmuls
    column-sum t into PSUM across all tiles.
  - The last e-tile is stored o-half-major so it streams and computes
    as two independent 256-column half-tiles: the post-stream tail is
    one short half-tile chain instead of a full-tile one.
"""
import numpy as np
from contextlib import ExitStack

D, B, N = 8, 8, 2048
NO = 512          # o columns per core
NE = 1024         # e rows per core
ET = NE // 128    # e-tiles per core
LT = ET - 1       # last tile (o-split)
N_CORES = 8

_NC = None


def _build():
    from concourse import bacc, tile, mybir, masks
    from concourse.alu_op_type import AluOpType as op

    f32 = mybir.dt.float32
    f16 = mybir.dt.float16
    i16 = mybir.dt.int16
    Copy = mybir.ActivationFunctionType.Copy

    nc = bacc.Bacc("TRN2", target_bir_lowering=False, debug=False)

    # Host-permuted layouts (see _in_maps): all loads are identity DMAs.
    dmap_d = nc.dram_tensor("dmap", (NE - 128, D, NO), f32, kind="ExternalInput")
    dmap7_d = nc.dram_tensor("dmap7", (128, 2, D, NO // 2), f32, kind="ExternalInput")
    xd_d = nc.dram_tensor("xd", (128, ET, D, B), f32, kind="ExternalInput")
    wl_d = nc.dram_tensor("wl", (NE - 128, B, NO), f32, kind="ExternalInput")
    wl7_d = nc.dram_tensor("wl7", (128, 2, B, NO // 2), f32, kind="ExternalInput")
    w_d = nc.dram_tensor("w", (128, ET, NO), f32, kind="ExternalInput")
    stdp_d = nc.dram_tensor("stdp", (128, ET, NO), f32, kind="ExternalInput")
    sgn_d = nc.dram_tensor("sgn", (128, ET, NO), f32, kind="ExternalInput")
    out_d = nc.dram_tensor("iout", (B, NO), f32, kind="ExternalOutput")

    with tile.TileContext(nc) as tc, ExitStack() as ctx:
        cpool = ctx.enter_context(tc.tile_pool(name="const", bufs=1))
        spool = ctx.enter_context(tc.tile_pool(name="stream", bufs=3))
        wpool = ctx.enter_context(tc.tile_pool(name="work", bufs=2))
        mpool = ctx.enter_context(tc.tile_pool(name="mpair", bufs=4))
        pspool = ctx.enter_context(tc.tile_pool(name="pst", bufs=2, space="PSUM"))
        accpool = ctx.enter_context(tc.tile_pool(name="acc", bufs=1, space="PSUM"))

        # ---- aux tensors first: the whole stream stays FIFO-busy and
        # A/C are ready before tile 0's elementwise work needs them.
        xd_sb = cpool.tile([128, ET, D, B], f32)
        nc.scalar.dma_start(xd_sb[:], xd_d[:])
        w_sb = cpool.tile([128, ET, NO], f16)
        nc.gpsimd.dma_start(w_sb[:], w_d[:])
        stdp_sb = cpool.tile([128, ET, NO], f16)
        nc.gpsimd.dma_start(stdp_sb[:], stdp_d[:])
        sgn_sb = cpool.tile([128, ET, NO], f16)
        nc.gpsimd.dma_start(sgn_sb[:], sgn_d[:])

        def load_tile(et):
            # f32->f16 casting DMAs must initiate from gpsimd (SWDGE).
            # dmap comes in d-halves so the PE select can start early.
            esl = slice(et * 128, (et + 1) * 128)
            dma_ = spool.tile([128, 4, NO], f16, name=f"dma_{et}", tag="dma")
            nc.gpsimd.dma_start(dma_[:], dmap_d[esl, 0:4])
            dmb = spool.tile([128, 4, NO], f16, name=f"dmb_{et}", tag="dmb")
            nc.gpsimd.dma_start(dmb[:], dmap_d[esl, 4:8])
            wl3 = spool.tile([128, B, NO], f16, name=f"wl3_{et}", tag="wl3")
            nc.gpsimd.dma_start(wl3[:], wl_d[esl])
            return dma_, dmb, wl3

        def load_tile7():
            halves = []
            for oh in range(2):
                dm7 = cpool.tile([128, D, NO // 2], f16, name=f"dm7_{oh}")
                nc.gpsimd.dma_start(dm7[:], dmap7_d[:, oh])
                wl7 = cpool.tile([128, B, NO // 2], f16, name=f"wl7_{oh}")
                nc.gpsimd.dma_start(wl7[:], wl7_d[:, oh])
                halves.append((dm7, wl7))
            return halves

        pre = {et: load_tile(et) for et in (0, 1, 2)}

        # ---- constants ------------------------------------------------
        ebs = []
        for b in range(B):
            ebt = cpool.tile([128, B], f16, name=f"eb{b}")
            nc.vector.memset(ebt[:], 0.0)
            nc.vector.memset(ebt[:, b:b + 1], 1.0)
            ebs.append(ebt)
        pw = cpool.tile([128, 1, 1, B], f32)
        for b in range(B):
            nc.vector.memset(pw[:, :, :, b], float(1 << b))
        ident3 = cpool.tile([128, D, 128], f16)
        for d in range(D):
            masks.make_identity(nc, ident3[:, d, :])

        # ---- pack Xd: packed16[e, et, d] = sum_b 2^b * Xd[d, b, e] ----
        xw = cpool.tile([128, ET, D, B], f32)
        nc.vector.tensor_tensor(
            xw[:], xd_sb[:], pw[:].broadcast_to((128, ET, D, B)), op=op.mult)
        packed = cpool.tile([128, ET, D], f32)
        nc.vector.tensor_reduce(
            packed[:], xw[:], axis=mybir.AxisListType.X, op=op.add)
        packed16 = cpool.tile([128, ET, D], f16)
        nc.vector.tensor_copy(packed16[:], packed[:])

        # ---- A = sgn*W*(1-f), C = sgn*f for all tiles (fp16) ----------
        omf = cpool.tile([128, ET, NO], f16)
        nc.scalar.activation(omf[:], stdp_sb[:], Copy, bias=1.0, scale=-1.0)
        C_sb = cpool.tile([128, ET, NO], f16)
        nc.vector.tensor_tensor(C_sb[:], sgn_sb[:], stdp_sb[:], op=op.mult)
        # reuse stdp_sb for sgn*W and A_sb out-of-place (data consumed)
        nc.vector.tensor_tensor(stdp_sb[:], sgn_sb[:], w_sb[:], op=op.mult)
        A_sb = cpool.tile([128, ET, NO], f16)
        nc.vector.tensor_tensor(A_sb[:], stdp_sb[:], omf[:], op=op.mult)

        acc = accpool.tile([B, NO], f32)

        def mk_dstack(et):
            ds = wpool.tile([128, D, 128], f16, tag="dstack")
            nc.vector.tensor_tensor(
                ds[:], ident3[:],
                packed16[:, et, :].unsqueeze(-1).broadcast_to((128, D, 128)),
                op=op.mult)
            return ds

        def stage_pi(ds, dms, now):
            pi_ps = pspool.tile([128, NO], f32, tag="pi_ps")
            piv = pi_ps[:, 0:now]
            d = 0
            for dmv in dms:
                for j in range(dmv.shape[1]):
                    nc.tensor.matmul(
                        piv, ds[:, d, :], dmv[:, j, :],
                        start=(d == 0), stop=(d == D - 1))
                    d += 1
            return pi_ps

        def stage_copy(pi_ps, now):
            pi_i = wpool.tile([128, NO], i16, tag="pi")
            nc.vector.tensor_copy(pi_i[:, 0:now], pi_ps[:, 0:now])
            return pi_i

        # Units: 7 full tiles then the two o-halves of the last tile.
        # Software pipeline: per iteration i, DVE runs shifts(i),
        # dstack(i+1), u/v(i), picopy(i+1), t-pairs(i); the PE runs
        # pi-matmuls(i+1) before acc-matmuls(i), so neither engine
        # stalls on the other tile's chain.
        units = [(et, slice(0, NO)) for et in range(LT)]
        units += [(LT, slice(oh * (NO // 2), (oh + 1) * (NO // 2)))
                  for oh in range(2)]
        NU = len(units)

        def unit_inputs(i):
            et, osl = units[i]
            if et < LT:
                dma_, dmb, wl3 = pre[et]
                return [dma_[:], dmb[:]], wl3[:]
            dm7, wl7 = pre[LT][osl.start // (NO // 2)]
            return [dm7[:]], wl7[:]

        ds0 = mk_dstack(0)
        dss = {0: ds0}
        dms0, _ = unit_inputs(0)
        pips = {0: stage_pi(ds0, dms0, NO)}
        piis = {0: stage_copy(pips[0], NO)}

        for i in range(NU):
            et, osl = units[i]
            now = osl.stop - osl.start
            dms, wlv = unit_inputs(i)
            pi_i = piis.pop(i)
            pips.pop(i, None)

            if et + 3 < LT and osl.start == 0:
                pre[et + 3] = load_tile(et + 3)
            elif et + 3 == LT and osl.start == 0:
                pre[LT] = load_tile7()

            # masks in b-pairs: DVE shifts feed i16->f16 casts on Scalar
            m_f = wpool.tile([128, B, NO], f16, tag="m_f")
            mis = []
            for k in range(4):
                m_i = mpool.tile([128, 2, NO], i16, tag="m_i")
                for j in range(2):
                    nc.vector.tensor_scalar(
                        m_i[:, j, 0:now], pi_i[:, 0:now], 2 * k + j, 1,
                        op0=op.logical_shift_right, op1=op.bitwise_and)
                mis.append(m_i)

            # next unit's delay-select runs on PE while this unit applies
            if i + 1 < NU:
                net, nosl = units[i + 1]
                nds = dss.get(net)
                if nds is None:
                    nds = mk_dstack(net)
                    dss = {net: nds}
                ndms, _ = unit_inputs(i + 1)
                pips[i + 1] = stage_pi(nds, ndms, nosl.stop - nosl.start)

            for k in range(4):
                nc.scalar.activation(
                    m_f[:, 2 * k:2 * k + 2, 0:now], mis[k][:, :, 0:now],
                    Copy)

            # u = C*Wl, v = u + A (out-of-place fp16)
            u = wpool.tile([128, B, NO], f16, tag="u")
            uv = u[:, :, 0:now]
            nc.vector.tensor_tensor(
                uv, wlv,
                C_sb[:, et, osl].unsqueeze(1).broadcast_to((128, B, now)),
                op=op.mult)
            v = wpool.tile([128, B, NO], f16, tag="v")
            vv = v[:, :, 0:now]
            nc.vector.tensor_tensor(
                vv, uv,
                A_sb[:, et, osl].unsqueeze(1).broadcast_to((128, B, now)),
                op=op.add)
            if i + 1 < NU:
                piis[i + 1] = stage_copy(pips[i + 1],
                                         units[i + 1][1].stop
                                         - units[i + 1][1].start)

            # t = v*m per b-pair (reuses u); column-sum each pair into
            # PSUM immediately so acc matmuls overlap the mask chain
            for k in range(4):
                bsl = slice(2 * k, 2 * k + 2)
                nc.vector.tensor_tensor(
                    u[:, bsl, 0:now], v[:, bsl, 0:now],
                    m_f[:, bsl, 0:now], op=op.mult)
                for b in (2 * k, 2 * k + 1):
                    nc.tensor.matmul(
                        acc[:, osl], ebs[b][:], u[:, b, 0:now],
                        start=(i == 0 and b == 0),
                        stop=(i == NU - 1 and b == B - 1),
                        skip_group_check=True)

        out_sb = cpool.tile([B, NO], f32)
        nc.vector.tensor_copy(out_sb[:], acc[:])
        nc.sync.dma_start(out_d[:], out_sb[:])

    nc.compile()
    return nc


def _in_maps(Xd, delaymap, W, Wlong, STDP_frac, signs):
    def emat(x):  # (NE, NO) slice -> (128, ET, NO): partition-major rows
        return np.ascontiguousarray(
            x.reshape(ET, 128, NO).transpose(1, 0, 2))

    maps = []
    for c in range(N_CORES):
        h, q = divmod(c, 4)
        e0, o0 = h * NE, q * NO
        es, os_ = slice(e0, e0 + NE), slice(o0, o0 + NO)
        e7 = slice(e0 + NE - 128, e0 + NE)
        xd_c = Xd[:, :, es].transpose(2, 0, 1)          # (NE, D, B)
        dm = delaymap[:, es, os_].transpose(1, 0, 2)    # (NE, D, NO)
        wl = Wlong[:, es, os_].transpose(1, 0, 2)       # (NE, B, NO)
        # last 128-e tile: o-half-major so it streams as 2 half-tiles
        dm7 = delaymap[:, e7, os_].transpose(1, 0, 2).reshape(
            128, D, 2, NO // 2).transpose(0, 2, 1, 3)   # (128, 2, D, 256)
        wl7 = Wlong[:, e7, os_].transpose(1, 0, 2).reshape(
            128, B, 2, NO // 2).transpose(0, 2, 1, 3)   # (128, 2, B, 256)
        maps.append({
            "dmap": np.ascontiguousarray(dm[:NE - 128]),
            "dmap7": np.ascontiguousarray(dm7),
            "xd": np.ascontiguousarray(
                xd_c.reshape(ET, 128, D, B).transpose(1, 0, 2, 3)),
            "wl": np.ascontiguousarray(wl[:NE - 128]),
            "wl7": np.ascontiguousarray(wl7),
            "w": emat(W[es, os_]),
            "stdp": emat(STDP_frac[es, os_]),
            "sgn": emat(signs[es, os_]),
        })
    return maps


def _gather(outs):
    return np.concatenate(
        [outs[q] + outs[q + 4] for q in range(4)], axis=1).astype(np.float32)


def kernel(Xd, delaymap, W, Wlong, STDP_frac, signs):
    global _NC
    from concourse.bass_utils import run_bass_kernel_spmd
    if _NC is None:
        _NC = _build()
    maps = _in_maps(Xd, delaymap, W, Wlong, STDP_frac, signs)
    res = run_bass_kernel_spmd(_NC, maps, list(range(N_CORES)))
    return _gather([r["iout"] for r in res.results])


# revision 11
# speedup vs baseline: 1.0901x; 1.0901x over previous
"""DeltaSynapse (gnn_message_passing) Trainium2 Bass kernel.

Computes I[b,o] = sum_e signs[e,o]*(W[e,o]*(1-f[e,o]) + Wlong[b,e,o]*f[e,o])
                  * Xpre[b,e,o],
with Xpre[b,e,o] = sum_d delaymap[d,e,o]*Xd[d,b,e]  (one-hot delay gather).

Strategy (8 NeuronCores): shard the postsynaptic axis o into 4 quarters of
512 and the presynaptic axis e into 2 halves of 1024; core (h,q) computes
the partial sum over its e-half for its o-quarter. The two e-half partials
are summed on the host (64KB) and the o-quarters concatenated.

On-device per core:
  - Xd is bit-packed once: packed[e,d] = sum_b 2^b * Xd[d,b,e] (PE transpose
    + weighted free-axis reduce).
  - Per e-tile (128 e's x 512 o's):
      Pi[e,o] = sum_d packed[e,d] * dmap[d,e,o]   (one-hot selection => Pi
      holds all 8 per-batch spike masks as an 8-bit integer, exact in fp16)
      m[b] = (uint8(Pi) >> b) & 1                  (per-batch masks)
      T[b] = (A + C*Wlong[b]) * m[b]               (A = sgn*W*(1-f), C=sgn*f)
      I[b,:] += column-sums of T[b] via PE matmul with a one-hot-column
      stationary matrix (lands each batch on its own PSUM partition).
  All bulk tensors are cast f32->f16 by the DMA engines on load; the
  e-reduction accumulates in fp32 PSUM.
"""
import numpy as np
from contextlib import ExitStack

D, B, N = 8, 8, 2048
NO = 512          # o columns per core
NE = 1024         # e rows per core
ET = NE // 128    # e-tiles per core
N_CORES = 8

_NC = None


def _build():
    from concourse import bacc, tile, mybir, masks
    from concourse.alu_op_type import AluOpType as op

    f32 = mybir.dt.float32
    f16 = mybir.dt.float16
    i16 = mybir.dt.int16

    nc = bacc.Bacc("TRN2", target_bir_lowering=False, debug=False)

    dmap_d = nc.dram_tensor("dmap", (NE, D, NO), f32, kind="ExternalInput")
    xd_d = nc.dram_tensor("xd", (D, B, NE), f32, kind="ExternalInput")
    wl_d = nc.dram_tensor("wl", (NE, B, NO), f32, kind="ExternalInput")
    w_d = nc.dram_tensor("w", (NE, NO), f32, kind="ExternalInput")
    stdp_d = nc.dram_tensor("stdp", (NE, NO), f32, kind="ExternalInput")
    sgn_d = nc.dram_tensor("sgn", (NE, NO), f32, kind="ExternalInput")
    out_d = nc.dram_tensor("iout", (B, NO), f32, kind="ExternalOutput")

    with tile.TileContext(nc) as tc, ExitStack() as ctx:
        cpool = ctx.enter_context(tc.tile_pool(name="const", bufs=1))
        pool = ctx.enter_context(tc.tile_pool(name="work", bufs=2))
        pspool = ctx.enter_context(tc.tile_pool(name="pst", bufs=2, space="PSUM"))
        accpool = ctx.enter_context(tc.tile_pool(name="acc", bufs=1, space="PSUM"))

        # ---- first two tiles' loads first: no slot waits exist yet, so
        # these issue immediately and DMA runs during the constant setup
        pre = {}
        for et in range(2):
            esl = slice(et * 128, (et + 1) * 128)
            dm3 = pool.tile([128, D, NO], f16, name=f"dm3_{et}", tag="dm3")
            nc.gpsimd.dma_start(dm3[:], dmap_d[esl])
            wl3 = pool.tile([128, B, NO], f16, name=f"wl3_{et}", tag="wl3")
            nc.gpsimd.dma_start(wl3[:], wl_d[esl])
            w_t = pool.tile([128, NO], f16, name=f"w_{et}", tag="w_t")
            nc.gpsimd.dma_start(w_t[:], w_d[esl, :])
            stdp_t = pool.tile([128, NO], f16, name=f"st_{et}", tag="stdp_t")
            nc.gpsimd.dma_start(stdp_t[:], stdp_d[esl, :])
            sgn_t = pool.tile([128, NO], f16, name=f"sg_{et}", tag="sgn_t")
            nc.gpsimd.dma_start(sgn_t[:], sgn_d[esl, :])
            pre[et] = (dm3, wl3, w_t, stdp_t, sgn_t)

        # ---- constants -------------------------------------------------
        ident = cpool.tile([D * B, D * B], f32)
        masks.make_identity(nc, ident[:])
        ebs = []
        for b in range(B):
            ebt = cpool.tile([128, B], f16, name=f"eb{b}")
            nc.vector.memset(ebt[:], 0.0)
            nc.vector.memset(ebt[:, b:b + 1], 1.0)
            ebs.append(ebt)
        pw = cpool.tile([128, D, B], f32)
        for b in range(B):
            nc.vector.memset(pw[:, :, b], float(1 << b))
        # stack of 8 identity matrices (f16) for building diag(packed[d])
        ident3 = cpool.tile([128, D, 128], f16)
        for d in range(D):
            masks.make_identity(nc, ident3[:, d, :])

        # ---- pack Xd: packed[e, et, d] = sum_b 2^b * Xd[d, b, e] -------
        xd_nat = cpool.tile([D * B, NE], f32)
        nc.sync.dma_start(xd_nat[:], xd_d[:].flatten_outer_dims())
        packed = cpool.tile([128, ET, D], f32)
        for c in range(ET):
            xdt_ps = pspool.tile([128, D * B], f32, name=f"xdt{c}", tag="xdt")
            nc.tensor.matmul(
                xdt_ps[:], xd_nat[:, c * 128:(c + 1) * 128], ident[:],
                is_transpose=True)
            xw = pool.tile([128, D, B], f32, name=f"xw{c}", tag="xw")
            nc.vector.tensor_tensor(
                xw[:], xdt_ps[:].rearrange("e (d b) -> e d b", d=D), pw[:],
                op=op.mult)
            nc.vector.tensor_reduce(
                packed[:, c, :], xw[:], axis=mybir.AxisListType.X, op=op.add)
        packed16 = cpool.tile([128, ET, D], f16)
        nc.vector.tensor_copy(packed16[:], packed[:])

        acc = accpool.tile([B, NO], f32)

        # ---- main loop over e-tiles ------------------------------------
        for et in range(ET):
            esl = slice(et * 128, (et + 1) * 128)

            if et in pre:
                dm3, wl3, w_t, stdp_t, sgn_t = pre[et]
            else:
                dm3 = pool.tile([128, D, NO], f16, tag="dm3")
                nc.gpsimd.dma_start(dm3[:], dmap_d[esl])
                wl3 = pool.tile([128, B, NO], f16, tag="wl3")
                nc.gpsimd.dma_start(wl3[:], wl_d[esl])
                w_t = pool.tile([128, NO], f16, tag="w_t")
                nc.gpsimd.dma_start(w_t[:], w_d[esl, :])
                stdp_t = pool.tile([128, NO], f16, tag="stdp_t")
                nc.gpsimd.dma_start(stdp_t[:], stdp_d[esl, :])
                sgn_t = pool.tile([128, NO], f16, tag="sgn_t")
                nc.gpsimd.dma_start(sgn_t[:], sgn_d[esl, :])

            # A = sgn*W*(1-f), C = sgn*f  (fp16)
            C_t = pool.tile([128, NO], f16, tag="C_t")
            nc.vector.tensor_tensor(C_t[:], sgn_t[:], stdp_t[:], op=op.mult)
            omf = pool.tile([128, NO], f16, tag="omf")
            nc.scalar.activation(
                omf[:], stdp_t[:], mybir.ActivationFunctionType.Copy,
                bias=1.0, scale=-1.0)
            sw = pool.tile([128, NO], f16, tag="sw")
            nc.vector.tensor_tensor(sw[:], sgn_t[:], w_t[:], op=op.mult)
            A_t = pool.tile([128, NO], f16, tag="A_t")
            nc.vector.tensor_tensor(A_t[:], sw[:], omf[:], op=op.mult)

            # Pi = sum_d diag(packed[:,et,d]) @ dmap[d] on the PE
            dstack = pool.tile([128, D, 128], f16, tag="dstack")
            nc.vector.tensor_tensor(
                dstack[:], ident3[:],
                packed16[:, et, :].unsqueeze(-1).broadcast_to((128, D, 128)),
                op=op.mult)
            pi_ps = pspool.tile([128, NO], f32, name=f"pi_ps{et}", tag="pi_ps")
            for d in range(D):
                nc.tensor.matmul(
                    pi_ps[:], dstack[:, d, :], dm3[:, d, :],
                    start=(d == 0), stop=(d == D - 1))
            pi_i16 = pool.tile([128, NO], i16, tag="pi_i16")
            nc.vector.tensor_copy(pi_i16[:], pi_ps[:])

            # masks m01 = (pi >> b) & 1 in i16 (no cast inside bitVec op);
            # one batched cast-copy to f16 on the scalar engine
            m_i16 = pool.tile([128, B, NO], i16, tag="m_i16")
            for b in range(B):
                nc.vector.tensor_scalar(
                    m_i16[:, b, :], pi_i16[:], b, 1,
                    op0=op.logical_shift_right, op1=op.bitwise_and)
            m_f16 = pool.tile([128, B, NO], f16, tag="m_f16")
            nc.scalar.activation(
                m_f16[:], m_i16[:], mybir.ActivationFunctionType.Copy)

            # T[b] = (A + C*Wlong[b]) * m[b], batched over b in 3D APs
            t_all = pool.tile([128, B, NO], f16, tag="t_all")
            nc.vector.tensor_tensor(
                t_all[:], wl3[:],
                C_t[:].unsqueeze(1).broadcast_to((128, B, NO)), op=op.mult)
            nc.vector.tensor_tensor(
                t_all[:], t_all[:],
                A_t[:].unsqueeze(1).broadcast_to((128, B, NO)), op=op.add)
            nc.vector.tensor_tensor(t_all[:], t_all[:], m_f16[:], op=op.mult)

            for b in range(B):
                nc.tensor.matmul(
                    acc[:], ebs[b][:], t_all[:, b, :],
                    start=(et == 0 and b == 0),
                    stop=(et == ET - 1 and b == B - 1))

        out_sb = cpool.tile([B, NO], f32)
        nc.vector.tensor_copy(out_sb[:], acc[:])
        nc.sync.dma_start(out_d[:], out_sb[:])

    nc.compile()
    return nc


def _in_maps(Xd, delaymap, W, Wlong, STDP_frac, signs):
    maps = []
    for c in range(N_CORES):
        h, q = divmod(c, 4)
        e0, o0 = h * NE, q * NO
        es, os_ = slice(e0, e0 + NE), slice(o0, o0 + NO)
        maps.append({
            "dmap": np.ascontiguousarray(
                delaymap[:, es, os_].transpose(1, 0, 2)),
            "xd": np.ascontiguousarray(Xd[:, :, es]),
            "wl": np.ascontiguousarray(
                Wlong[:, es, os_].transpose(1, 0, 2)),
            "w": np.ascontiguousarray(W[es, os_]),
            "stdp": np.ascontiguousarray(STDP_frac[es, os_]),
            "sgn": np.ascontiguousarray(signs[es, os_]),
        })
    return maps


def _gather(outs):
    return np.concatenate(
        [outs[q] + outs[q + 4] for q in range(4)], axis=1).astype(np.float32)


def kernel(Xd, delaymap, W, Wlong, STDP_frac, signs):
    global _NC
    from concourse.bass_utils import run_bass_kernel_spmd
    if _NC is None:
        _NC = _build()
    maps = _in_maps(Xd, delaymap, W, Wlong, STDP_frac, signs)
    res = run_bass_kernel_spmd(_NC, maps, list(range(N_CORES)))
    return _gather([r["iout"] for r in res.results])

